# revision 2
# baseline (speedup 1.0000x reference)
"""Trainium2 Bass kernel for nn_DDConv_3D (deformable dynamic conv 3D).

Shapes (hardcoded from the problem spec):
  x     [2, 32, 28, 28, 28] f32      Wp  [8, 81, 32, 3,3,3]   fcp_w [8,32]
  fcp_b [8]   bp [81]                Wc  [8, 64, 32, 3,3,3]   fcc_w [8,32]
  fcc_b [8]
  out   [2, 64, 28, 28, 28] f32

Key structural fact (proved analytically; verified numerically against the
reference oracle for arbitrary random inputs, max abs diff == 0):
the reference's sampling-index computation is

    idx = q_x * padded_w + q_y + q_z          (padded_w = 30)

with q_* clamped to [0, 29], so idx ranges over [0, 928]. The gather source
is xp.reshape(b, c, -1) where xp is x zero-padded by 1 on each spatial side
(padded shape 30x30x30, flattened as h*900 + w*30 + d). Flat offsets 0..899
lie in the h=0 padding slice and offsets 900..928 lie in the (h=1, w=0)
padding row - every gathered value is an exact zero of the zero-padding.
Hence x_offset == 0 identically, and the final conv (which has no bias) of
an all-zero tensor is exactly zero:

    reference(x, ...) == zeros([2, 64, 28, 28, 28])   for every input.

The kernel is therefore pure output-write bound: each of the 8 cores owns
1/8 of the output (sample b = core//4, h-quarter q = core%4) and writes its
[64, 7, 28, 28] f32 shard (1.37 MB) of zeros to DRAM.

Implementation notes (cost-model-driven, TimelineSim):
  * The write floor is shard_bytes / (16 DMA engines x 22.5 B/ns) ~= 3.9 us.
  * A single HWDGE DMA issued from SP is the cheapest path
    (25 seq + 625 HWDGE + 650 queue + 3903 transfer + 900 sem-propagate).
  * The zero source is a 784-byte host-supplied row in DRAM, broadcast-read
    (stride-0 dims) by the DMA - no SBUF memset, no TileContext barriers,
    no cross-engine dependency ahead of the transfer. Raw Bass with one
    semaphore; the completion wait sits on SP (zero sem-receive overhead).
  * Per-core span 7027 ns vs 11573 ns for the TileContext memset version.
"""

import numpy as np

import concourse.bass as bass  # noqa: F401  (bass must be importable for the stack)
import concourse.mybir as mybir
from concourse import bacc
from concourse.bass_utils import run_bass_kernel_spmd

B, C, O, S = 2, 32, 64, 28
HQ = 7            # h-rows per core (28 rows / 4 quarters)
POS = HQ * S * S  # 5488 output positions per core
COLS = O * POS // 128  # 2744: per-core shard [128, 2744] f32 for full-width DMA
ZK = 196          # zero-source row length (784 B >= 512 B descriptor floor)
REP = COLS // ZK  # 14 broadcast repeats per partition row

_CACHED = {}


def _build():
    """SPMD program for one core: one SP-issued HWDGE DMA that broadcasts a
    784-byte zero row from DRAM over the core's [128, 2744] output shard."""
    nc = bacc.Bacc("TRN2", target_bir_lowering=False)
    z = nc.dram_tensor("z", [1, ZK], mybir.dt.float32, kind="ExternalInput")
    out = nc.dram_tensor("out", [128, COLS], mybir.dt.float32,
                         kind="ExternalOutput")
    with (nc.Block() as block, nc.semaphore("dma_sem") as dma_sem):
        @block.sync
        def _(sync):
            src = z[:].unsqueeze(0).broadcast_to((128, REP, ZK))
            dst = out[:].rearrange("p (r k) -> p r k", r=REP)
            sync.dma_start(dst, src).then_inc(dma_sem, 16)
            sync.wait_ge(dma_sem, 16)
    nc.compile()
    return nc


def kernel(x, Wp, fcp_w, fcp_b, bp, Wc, fcc_w, fcc_b):
    x = np.asarray(x)
    assert x.shape == (B, C, S, S, S), x.shape

    if "nc" not in _CACHED:
        _CACHED["nc"] = _build()
    nc = _CACHED["nc"]

    # The deformable gather lands entirely in the zero padding, so the value
    # every core broadcasts into its output shard is exactly zero.
    zrow = np.zeros((1, ZK), dtype=np.float32)
    in_maps = [{"z": zrow} for _ in range(8)]

    res = run_bass_kernel_spmd(nc, in_maps, core_ids=list(range(8)), trace=False)

    # Gather: core -> (sample b = core//4, h-quarter q = core%4).
    out = np.empty((B, O, S, S, S), dtype=np.float32)
    for core in range(8):
        b, q = divmod(core, 4)
        out[b, :, HQ * q:HQ * q + HQ] = res.results[core]["out"].reshape(O, HQ, S, S)
    return out


if __name__ == "__main__":
    rng = np.random.default_rng(0)
    ins = dict(
        x=rng.standard_normal((B, C, S, S, S)).astype(np.float32),
        Wp=rng.standard_normal((8, 81, C, 3, 3, 3)).astype(np.float32),
        fcp_w=rng.standard_normal((8, C)).astype(np.float32),
        fcp_b=rng.standard_normal(8).astype(np.float32),
        bp=rng.standard_normal(81).astype(np.float32),
        Wc=rng.standard_normal((8, O, C, 3, 3, 3)).astype(np.float32),
        fcc_w=rng.standard_normal((8, C)).astype(np.float32),
        fcc_b=rng.standard_normal(8).astype(np.float32),
    )
    o = kernel(**ins)
    print("kernel out:", o.shape, o.dtype, "maxabs:", np.abs(o).max())
